# revision 5
# baseline (speedup 1.0000x reference)
"""Trainium2 kernel for ApplyStickerLayer: out = roll(subimg, (80,80), (2,3)) + base_image.

Input structure (guaranteed by the layer): subimg is zero outside the 50x50
sticker at the origin, and base_image is zero inside the 50x50 destination
window at (80,80).  The roll therefore just moves the sticker into the window:

    out[b] = base            everywhere except the window [80:130, 80:130]
    out[b][win] = subimg[b][0:50, 0:50] + base[win]

Pure data parallel across 8 NeuronCores (32 batches per core).  Per core the
NEFF writes each (b, c) channel image as four disjoint DRAM regions in the
flattened 50176-element channel space:

    top   [0, 18000)                 base broadcast (stride-0 source DMA)
    diag  49 x 174 strided chunks    base broadcast   (rows 80..129 minus window)
    tail  [29026, 50176)             base broadcast
    win   50 x 50                    sticker + base-window (via SBUF add)

All regions are disjoint so there are no write-ordering hazards.  Traffic per
core: ~19.3 MB written, ~1.6 MB read -> memory-roofline ~55-60 us.
"""

import os
import sys

import numpy as np

if "/opt/trn_rl_repo" not in sys.path:
    sys.path.insert(0, "/opt/trn_rl_repo")

import concourse.bacc as bacc
import concourse.bass as bass
import concourse.mybir as mybir
import concourse.tile as tile
from concourse.bass_utils import run_bass_kernel_spmd

N_CORES = 8
B, C, H, W = 256, 3, 224, 224
BS = B // N_CORES  # 32 batches per core
SH, SW = 80, 80  # roll shift == window origin
KH, KW = 50, 50  # sticker size

CHW = H * W  # 50176 elements per channel image
IMG = C * CHW  # 150528 elements per batch image

TOP_LEN = SH * W + SW  # 18000: rows 0..79 + row 80 cols 0..79
DIAG_OFF = TOP_LEN + KW  # 18050: (80, 130)
DIAG_ROWS = KH - 1  # 49 chunks
DIAG_LEN = W - KW  # 174: (h, 130:224) + (h+1, 0:80)
TAIL_OFF = DIAG_OFF + (DIAG_ROWS - 1) * W + DIAG_LEN + KW  # 29026: (129, 130)
TAIL_LEN = CHW - TAIL_OFF  # 21150

_F32 = mybir.dt.float32

DEFAULT_CFG = {
    "top": (125, 144),  # p x f factorization of TOP_LEN for the SBUF tile
    "tail": (94, 225),  # p x f factorization of TAIL_LEN
    "bcast": True,  # stride-0 (broadcast) source DMAs over the batch dim
    "b_split": 1,  # split each broadcast store into this many DMAs over b
}


def build_nc(cfg=None):
    cfg = {**DEFAULT_CFG, **(cfg or {})}
    top_p, top_f = cfg["top"]
    tail_p, tail_f = cfg["tail"]
    assert top_p * top_f == TOP_LEN and tail_p * tail_f == TAIL_LEN

    nc = bacc.Bacc("TRN2", target_bir_lowering=False, num_devices=N_CORES)
    sub = nc.declare_dram_parameter("subimg", [BS, C, H, W], _F32, isOutput=False)
    base = nc.declare_dram_parameter("base", [C, H, W], _F32, isOutput=False)
    out = nc.declare_dram_parameter("out", [BS, C, H, W], _F32, isOutput=True)

    base_flat = base[:, :, :].rearrange("c h w -> (c h w)")
    out_bflat = out[:, :, :, :].rearrange("b c h w -> b (c h w)")

    with tile.TileContext(nc) as tc:
        with (
            tc.tile_pool(name="consts", bufs=1) as cpool,
            tc.tile_pool(name="work", bufs=1) as wpool,
        ):
            # ---- preload the per-channel base pieces ----
            piece_tiles = {}
            for c in range(C):
                o = c * CHW
                t_top = cpool.tile([top_p, top_f], _F32, tag=f"top{c}")
                nc.sync.dma_start(
                    out=t_top[:, :],
                    in_=base_flat[o : o + TOP_LEN].rearrange("(p f) -> p f", p=top_p),
                )
                t_diag = cpool.tile([DIAG_ROWS, DIAG_LEN], _F32, tag=f"diag{c}")
                nc.sync.dma_start(
                    out=t_diag[:, :],
                    in_=bass.AP(base, o + DIAG_OFF, [[W, DIAG_ROWS], [1, DIAG_LEN]]),
                )
                t_tail = cpool.tile([tail_p, tail_f], _F32, tag=f"tail{c}")
                nc.sync.dma_start(
                    out=t_tail[:, :],
                    in_=base_flat[o + TAIL_OFF : o + CHW].rearrange(
                        "(p f) -> p f", p=tail_p
                    ),
                )
                piece_tiles[c] = (t_top, t_diag, t_tail)

            # ---- window path: win = sticker + base_window, all via DMA ----
            t_bwin = cpool.tile([KH, C * KW], _F32, tag="bwin")
            nc.sync.dma_start(
                out=t_bwin[:, :].rearrange("p (c w) -> p c w", c=C),
                in_=base[:, SH : SH + KH, SW : SW + KW].rearrange("c h w -> h c w"),
            )
            # replicate base window across the batch dim (SBUF -> SBUF)
            t_win = wpool.tile([KH, BS * C * KW], _F32, tag="win")
            nc.sync.dma_start(
                out=t_win[:, :].rearrange("p (b cw) -> p b cw", b=BS),
                in_=t_bwin[:, :].unsqueeze(1).broadcast_to([KH, BS, C * KW]),
            )
            # accumulate the sticker into it during the load (SWDGE accum)
            nc.gpsimd.dma_start(
                out=t_win[:, :].rearrange("p (bc w) -> p bc w", w=KW),
                in_=sub[:, :, 0:KH, 0:KW].rearrange("b c h w -> h b c w"),
                accum_op=mybir.AluOpType.add,
            )
            nc.sync.dma_start(
                out=out[:, :, SH : SH + KH, SW : SW + KW].rearrange(
                    "b c h w -> h b c w"
                ),
                in_=t_win[:, :].rearrange("p (bc w) -> p bc w", w=KW),
            )

            # ---- replicated base stores ----
            nb = BS // cfg["b_split"]
            for c in range(C):
                t_top, t_diag, t_tail = piece_tiles[c]
                o = c * CHW
                for s in range(cfg["b_split"]):
                    b0 = s * nb
                    if cfg["bcast"]:
                        # source repeats each SBUF partition nb times (step-0
                        # middle dim); dest walks (p, b, f) to match.
                        nc.sync.dma_start(
                            out=out_bflat[b0 : b0 + nb, o : o + TOP_LEN].rearrange(
                                "b (p f) -> p b f", p=top_p
                            ),
                            in_=t_top[:, :].unsqueeze(1).broadcast_to(
                                [top_p, nb, top_f]
                            ),
                        )
                        nc.sync.dma_start(
                            out=bass.AP(
                                out,
                                b0 * IMG + o + DIAG_OFF,
                                [[W, DIAG_ROWS], [IMG, nb], [1, DIAG_LEN]],
                            ),
                            in_=t_diag[:, :].unsqueeze(1).broadcast_to(
                                [DIAG_ROWS, nb, DIAG_LEN]
                            ),
                        )
                        nc.sync.dma_start(
                            out=out_bflat[b0 : b0 + nb, o + TAIL_OFF : o + CHW].rearrange(
                                "b (p f) -> p b f", p=tail_p
                            ),
                            in_=t_tail[:, :].unsqueeze(1).broadcast_to(
                                [tail_p, nb, tail_f]
                            ),
                        )
                    else:
                        for b in range(b0, b0 + nb):
                            nc.sync.dma_start(
                                out=out_bflat[b, o : o + TOP_LEN].rearrange(
                                    "(p f) -> p f", p=top_p
                                ),
                                in_=t_top[:, :],
                            )
                            nc.sync.dma_start(
                                out=bass.AP(
                                    out,
                                    b * IMG + o + DIAG_OFF,
                                    [[W, DIAG_ROWS], [1, DIAG_LEN]],
                                ),
                                in_=t_diag[:, :],
                            )
                            nc.sync.dma_start(
                                out=out_bflat[b, o + TAIL_OFF : o + CHW].rearrange(
                                    "(p f) -> p f", p=tail_p
                                ),
                                in_=t_tail[:, :],
                            )
    nc.compile()
    return nc


def run(inputs, cfg=None, trace=False, **kw):
    sub = np.ascontiguousarray(inputs["subimg"], dtype=np.float32)
    basei = np.ascontiguousarray(inputs["base_image"], dtype=np.float32)
    assert sub.shape == (B, C, H, W) and basei.shape == (1, C, H, W)

    nc = build_nc(cfg)
    in_maps = [
        {"subimg": sub[i * BS : (i + 1) * BS], "base": basei[0]}
        for i in range(N_CORES)
    ]
    res = run_bass_kernel_spmd(nc, in_maps, list(range(N_CORES)), trace=trace, **kw)
    full = np.concatenate(
        [res.results[i]["out"] for i in range(N_CORES)], axis=0
    ).astype(np.float32, copy=False)
    return full, res


def kernel(**inputs) -> np.ndarray:
    out, _ = run(inputs)
    return out


# revision 8
# speedup vs baseline: 1.1041x; 1.1041x over previous
"""Trainium2 kernel for ApplyStickerLayer: out = roll(subimg, (80,80), (2,3)) + base_image.

Input structure (guaranteed by the layer): subimg is zero outside the 50x50
sticker at the origin, and base_image is zero inside the 50x50 destination
window at (80,80).  The roll therefore just moves the sticker into the window:

    out[b] = base            everywhere except the window [80:130, 80:130]
    out[b][win] = subimg[b][0:50, 0:50] + base[win]

Pure data parallel across 8 NeuronCores (32 batches per core).  Per core the
NEFF writes each (b, c) channel image as four disjoint DRAM regions in the
flattened 50176-element channel space:

    top   [0, 18000)                 base broadcast (stride-0 source DMA)
    diag  49 x 174 strided chunks    base broadcast   (rows 80..129 minus window)
    tail  [29026, 50176)             base broadcast
    win   50 x 50                    sticker + base-window (via SBUF add)

All regions are disjoint so there are no write-ordering hazards.  Traffic per
core: ~19.3 MB written, ~1.6 MB read -> memory-roofline ~55-60 us.
"""

import os
import sys

import numpy as np

if "/opt/trn_rl_repo" not in sys.path:
    sys.path.insert(0, "/opt/trn_rl_repo")

import concourse.bacc as bacc
import concourse.bass as bass
import concourse.mybir as mybir
import concourse.tile as tile
from concourse.bass_utils import run_bass_kernel_spmd

N_CORES = 8
B, C, H, W = 256, 3, 224, 224
BS = B // N_CORES  # 32 batches per core
SH, SW = 80, 80  # roll shift == window origin
KH, KW = 50, 50  # sticker size

CHW = H * W  # 50176 elements per channel image
IMG = C * CHW  # 150528 elements per batch image

TOP_LEN = SH * W + SW  # 18000: rows 0..79 + row 80 cols 0..79
DIAG_OFF = TOP_LEN + KW  # 18050: (80, 130)
DIAG_ROWS = KH - 1  # 49 chunks
DIAG_LEN = W - KW  # 174: (h, 130:224) + (h+1, 0:80)
TAIL_OFF = DIAG_OFF + (DIAG_ROWS - 1) * W + DIAG_LEN + KW  # 29026: (129, 130)
TAIL_LEN = CHW - TAIL_OFF  # 21150

_F32 = mybir.dt.float32

DEFAULT_CFG = {
    "top": (125, 144),  # p x f factorization of TOP_LEN for the SBUF tile
    "tail": (94, 225),  # p x f factorization of TAIL_LEN
    "bcast": True,  # stride-0 (broadcast) source DMAs over the batch dim
    "b_split": 1,  # split each broadcast store into this many DMAs over b
    "eng_pre": "gpsimd",  # engine for base-piece preloads
    "eng_win": "gpsimd",  # engine for window path (rep / store; accum is always gpsimd)
    "eng_st": "gpsimd",  # engine for the replicated base stores
    "st_rings": None,  # e.g. ["sync","scalar"]: round-robin stores over rings
}


def build_nc(cfg=None):
    cfg = {**DEFAULT_CFG, **(cfg or {})}
    top_p, top_f = cfg["top"]
    tail_p, tail_f = cfg["tail"]
    assert top_p * top_f == TOP_LEN and tail_p * tail_f == TAIL_LEN

    nc = bacc.Bacc("TRN2", target_bir_lowering=False, num_devices=N_CORES)
    sub = nc.declare_dram_parameter("subimg", [BS, C, H, W], _F32, isOutput=False)
    base = nc.declare_dram_parameter("base", [C, H, W], _F32, isOutput=False)
    out = nc.declare_dram_parameter("out", [BS, C, H, W], _F32, isOutput=True)

    eng_pre = getattr(nc, cfg["eng_pre"])
    eng_win = getattr(nc, cfg["eng_win"])
    if cfg["st_rings"]:
        _rings = [getattr(nc, r) for r in cfg["st_rings"]]
        _ring_i = [0]

        class _RR:
            def dma_start(self, *a, **k):
                e = _rings[_ring_i[0] % len(_rings)]
                _ring_i[0] += 1
                return e.dma_start(*a, **k)

        eng_st = _RR()
    else:
        eng_st = getattr(nc, cfg["eng_st"])

    base_flat = base[:, :, :].rearrange("c h w -> (c h w)")
    out_bflat = out[:, :, :, :].rearrange("b c h w -> b (c h w)")

    with tile.TileContext(nc) as tc:
        with (
            tc.tile_pool(name="consts", bufs=1) as cpool,
            tc.tile_pool(name="work", bufs=1) as wpool,
        ):
            # ---- preload the per-channel base pieces ----
            piece_tiles = {}
            for c in range(C):
                o = c * CHW
                t_top = cpool.tile([top_p, top_f], _F32, tag=f"top{c}")
                eng_pre.dma_start(
                    out=t_top[:, :],
                    in_=base_flat[o : o + TOP_LEN].rearrange("(p f) -> p f", p=top_p),
                )
                t_diag = cpool.tile([DIAG_ROWS, DIAG_LEN], _F32, tag=f"diag{c}")
                eng_pre.dma_start(
                    out=t_diag[:, :],
                    in_=bass.AP(base, o + DIAG_OFF, [[W, DIAG_ROWS], [1, DIAG_LEN]]),
                )
                t_tail = cpool.tile([tail_p, tail_f], _F32, tag=f"tail{c}")
                eng_pre.dma_start(
                    out=t_tail[:, :],
                    in_=base_flat[o + TAIL_OFF : o + CHW].rearrange(
                        "(p f) -> p f", p=tail_p
                    ),
                )
                piece_tiles[c] = (t_top, t_diag, t_tail)

            # ---- window path: win = sticker + base_window, all via DMA ----
            t_bwin = cpool.tile([KH, C * KW], _F32, tag="bwin")
            eng_pre.dma_start(
                out=t_bwin[:, :].rearrange("p (c w) -> p c w", c=C),
                in_=base[:, SH : SH + KH, SW : SW + KW].rearrange("c h w -> h c w"),
            )
            # replicate base window across the batch dim (SBUF -> SBUF)
            t_win = wpool.tile([KH, BS * C * KW], _F32, tag="win")
            eng_win.dma_start(
                out=t_win[:, :].rearrange("p (b cw) -> p b cw", b=BS),
                in_=t_bwin[:, :].unsqueeze(1).broadcast_to([KH, BS, C * KW]),
            )
            # accumulate the sticker into it during the load (SWDGE accum)
            nc.gpsimd.dma_start(
                out=t_win[:, :].rearrange("p (bc w) -> p bc w", w=KW),
                in_=sub[:, :, 0:KH, 0:KW].rearrange("b c h w -> h b c w"),
                accum_op=mybir.AluOpType.add,
            )
            eng_win.dma_start(
                out=out[:, :, SH : SH + KH, SW : SW + KW].rearrange(
                    "b c h w -> h b c w"
                ),
                in_=t_win[:, :].rearrange("p (bc w) -> p bc w", w=KW),
            )

            # ---- replicated base stores ----
            nb = BS // cfg["b_split"]
            for c in range(C):
                t_top, t_diag, t_tail = piece_tiles[c]
                o = c * CHW
                for s in range(cfg["b_split"]):
                    b0 = s * nb
                    if cfg["bcast"]:
                        # source repeats each SBUF partition nb times (step-0
                        # middle dim); dest walks (p, b, f) to match.
                        eng_st.dma_start(
                            out=out_bflat[b0 : b0 + nb, o : o + TOP_LEN].rearrange(
                                "b (p f) -> p b f", p=top_p
                            ),
                            in_=t_top[:, :].unsqueeze(1).broadcast_to(
                                [top_p, nb, top_f]
                            ),
                        )
                        eng_st.dma_start(
                            out=bass.AP(
                                out,
                                b0 * IMG + o + DIAG_OFF,
                                [[W, DIAG_ROWS], [IMG, nb], [1, DIAG_LEN]],
                            ),
                            in_=t_diag[:, :].unsqueeze(1).broadcast_to(
                                [DIAG_ROWS, nb, DIAG_LEN]
                            ),
                        )
                        eng_st.dma_start(
                            out=out_bflat[b0 : b0 + nb, o + TAIL_OFF : o + CHW].rearrange(
                                "b (p f) -> p b f", p=tail_p
                            ),
                            in_=t_tail[:, :].unsqueeze(1).broadcast_to(
                                [tail_p, nb, tail_f]
                            ),
                        )
                    else:
                        for b in range(b0, b0 + nb):
                            eng_st.dma_start(
                                out=out_bflat[b, o : o + TOP_LEN].rearrange(
                                    "(p f) -> p f", p=top_p
                                ),
                                in_=t_top[:, :],
                            )
                            eng_st.dma_start(
                                out=bass.AP(
                                    out,
                                    b * IMG + o + DIAG_OFF,
                                    [[W, DIAG_ROWS], [1, DIAG_LEN]],
                                ),
                                in_=t_diag[:, :],
                            )
                            eng_st.dma_start(
                                out=out_bflat[b, o + TAIL_OFF : o + CHW].rearrange(
                                    "(p f) -> p f", p=tail_p
                                ),
                                in_=t_tail[:, :],
                            )
    nc.compile()
    return nc


def run(inputs, cfg=None, trace=False, **kw):
    sub = np.ascontiguousarray(inputs["subimg"], dtype=np.float32)
    basei = np.ascontiguousarray(inputs["base_image"], dtype=np.float32)
    assert sub.shape == (B, C, H, W) and basei.shape == (1, C, H, W)

    nc = build_nc(cfg)
    in_maps = [
        {"subimg": sub[i * BS : (i + 1) * BS], "base": basei[0]}
        for i in range(N_CORES)
    ]
    res = run_bass_kernel_spmd(nc, in_maps, list(range(N_CORES)), trace=trace, **kw)
    full = np.concatenate(
        [res.results[i]["out"] for i in range(N_CORES)], axis=0
    ).astype(np.float32, copy=False)
    return full, res


def kernel(**inputs) -> np.ndarray:
    out, _ = run(inputs)
    return out
